# revision 2
# baseline (speedup 1.0000x reference)
"""Trainium2 Bass kernel v2 for nn_Encoder_meta (GRU with per-step meta transform).

Reference computation (per batch row b, over T steps):
    x_cat = concat(x_l, x_t, x_w, x_s)            # [B, T, 160]
    x' = tanh(h @ Wt_h.T + x_cat_t @ Wt_x.T + bt) # [B, 256]
    gx = x' @ W_ih.T + b_ih ; gh = h @ W_hh.T + b_hh
    r = sig(gxr+ghr); z = sig(gxz+ghz); n = tanh(gxn + r*ghn)
    h = (1-z)*n + z*h
Output: h_T as [1, B, H].

Strategy: data-parallel over batch across 8 cores (32 rows each).
v2 redesign vs baseline: few large-N matmuls instead of many small ones.
Per step:
  - gates psum g [128, 512]: partition 32j+b packs (hidden-block j, batch b),
    cols = gxn|r|z|ghn segments of 128. One K=4 bias matmul initializes the
    bank with all biases (rank-4 delta trick), then 16 h-matmuls and 8
    x'-matmuls with N=384 accumulate gx+gh in one group.
  - x'pre psum [32, 256] from 6 matmuls (x streamed from SBUF-resident xT,
    eliminating the u-precompute pass; bt folded via ones row of xT).
  - h -> hT via DVE 32x32 block transpose (weights host-permuted to match),
    keeping the PE free and psum untouched.
"""

import numpy as np
import ml_dtypes
from contextlib import ExitStack

import concourse.bass as bass
import concourse.mybir as mybir
import concourse.tile as tile
from concourse import bacc
from concourse.bass_utils import run_bass_kernel_spmd

F32 = mybir.dt.float32
F32R = mybir.dt.float32r
BF16 = mybir.dt.bfloat16
BF = ml_dtypes.bfloat16

B, T, H, XP, D = 256, 512, 512, 256, 160
DA = D + 1  # ones row folded in for bt
NCORES, BC = 8, 32
GATE3 = 3 * H
TSTEPS = T  # loop steps actually executed (monkeypatchable for overhead tests)

Act = mybir.ActivationFunctionType


def _build_nc():
    nc = bacc.Bacc("TRN2", target_bir_lowering=False, debug=False, num_devices=NCORES)

    xT = nc.dram_tensor("xT", [DA, T * BC], BF16, kind="ExternalInput")
    wtxa = nc.dram_tensor("wtxa", [128, XP], BF16, kind="ExternalInput")
    wtxb = nc.dram_tensor("wtxb", [DA - 128, XP], BF16, kind="ExternalInput")
    wth = nc.dram_tensor("wth", [128, 4 * XP], BF16, kind="ExternalInput")
    whh = nc.dram_tensor("whh", [128, 4 * GATE3], BF16, kind="ExternalInput")
    wih = nc.dram_tensor("wih", [128, 2 * GATE3], BF16, kind="ExternalInput")
    blhs = nc.dram_tensor("blhs", [4, 128], F32R, kind="ExternalInput")
    brhs = nc.dram_tensor("brhs", [4, 512], F32R, kind="ExternalInput")
    hout = nc.dram_tensor("hout", [128, 128], BF16, kind="ExternalOutput")

    with tile.TileContext(nc) as tc:
        _kernel_body(tc, xT, wtxa, wtxb, wth, whh, wih, blhs, brhs, hout)
    nc.compile()
    return nc


def _kernel_body(tc, xT, wtxa, wtxb, wth, whh, wih, blhs, brhs, hout):
    nc = tc.nc
    with ExitStack() as ctx:
        const = ctx.enter_context(tc.tile_pool(name="const", bufs=1))

        wtxa_sb = const.tile([128, XP], BF16)
        nc.sync.dma_start(out=wtxa_sb[:], in_=wtxa.ap())
        wtxb_sb = const.tile([DA - 128, XP], BF16)
        nc.sync.dma_start(out=wtxb_sb[:], in_=wtxb.ap())
        wth_sb = const.tile([128, 4 * XP], BF16)
        nc.sync.dma_start(out=wth_sb[:], in_=wth.ap())
        whh_sb = const.tile([128, 4 * GATE3], BF16)
        nc.sync.dma_start(out=whh_sb[:], in_=whh.ap())
        wih_sb = const.tile([128, 2 * GATE3], BF16)
        nc.sync.dma_start(out=wih_sb[:], in_=wih.ap())
        blhs_sb = const.tile([4, 128], F32R)
        nc.sync.dma_start(out=blhs_sb[:], in_=blhs.ap())
        brhs_sb = const.tile([4, 512], F32R)
        nc.sync.dma_start(out=brhs_sb[:], in_=brhs.ap())

        identb = const.tile([32, 32], BF16)
        from concourse.masks import make_identity

        make_identity(nc, identb[:])

        # x data resident in SBUF, loaded in chunks so step 0 starts early
        xa_sb = const.tile([128, T * BC], BF16)
        xb_sb = const.tile([DA - 128, T * BC], BF16)
        NDMA = 8
        CHW = (T * BC) // NDMA
        for c in range(NDMA):
            nc.sync.dma_start(
                out=xa_sb[:, CHW * c : CHW * (c + 1)],
                in_=xT.ap()[0:128, CHW * c : CHW * (c + 1)],
            )
            nc.sync.dma_start(
                out=xb_sb[:, CHW * c : CHW * (c + 1)],
                in_=xT.ap()[128:DA, CHW * c : CHW * (c + 1)],
            )

        h_sb = const.tile([128, 128], BF16)  # h_sb[32j+b, 32q+r] = h[b, 128j+32q+r]
        hT_sb = const.tile([128, 128], BF16)  # block-transposed h (see _prep perm)
        nc.vector.memset(h_sb[:], 0.0)
        nc.vector.memset(hT_sb[:], 0.0)

        gps = ctx.enter_context(tc.tile_pool(name="gps", bufs=2, space="PSUM"))
        xps = ctx.enter_context(tc.tile_pool(name="xps", bufs=2, space="PSUM"))
        tps = ctx.enter_context(tc.tile_pool(name="tps", bufs=2, space="PSUM"))
        work = ctx.enter_context(tc.tile_pool(name="work", bufs=2))

        mm = nc.tensor.matmul

        for t in range(TSTEPS):
            # ---- gates psum [128, 512]: cols gxn|r|z|ghn, partition 32j+b ----
            g = gps.tile([128, 512], F32, tag="g")
            # bias init: rank-4 delta matmul fills g[32j+b, n] = bias(j, n)
            mm(g[:], blhs_sb[:], brhs_sb[:], start=True, stop=False,
               skip_group_check=True)

            # ---- x'pre psum [32, 256] (full 512-col bank to own the region) --
            px = xps.tile([32, 512], F32, tag="px")
            pxv = px[:, 0:XP]
            mm(pxv, xa_sb[:, BC * t : BC * (t + 1)], wtxa_sb[:],
               start=True, stop=False, skip_group_check=True)
            mm(pxv, xb_sb[:, BC * t : BC * (t + 1)], wtxb_sb[:],
               start=False, stop=False, skip_group_check=True)
            for c in range(4):
                mm(pxv, hT_sb[:, 32 * c : 32 * (c + 1)],
                   wth_sb[:, XP * c : XP * (c + 1)],
                   start=False, stop=(c == 3), skip_group_check=True)

            # ---- 16 gate h-matmuls: cols 128..511 = r|z|ghn ----
            for c in range(4):
                for j in range(4):
                    mm(g[32 * j : 32 * (j + 1), 128:512],
                       hT_sb[:, 32 * c : 32 * (c + 1)],
                       whh_sb[:, GATE3 * c + 384 * j : GATE3 * c + 384 * (j + 1)],
                       start=False, stop=False, skip_group_check=True,
                       tile_position=(0, 32 * j))

            # ---- x' = tanh(px), transpose to [x'dim, b] for stationary use --
            xp_s = work.tile([32, XP], BF16, tag="xp_s")
            nc.scalar.activation(xp_s[:], pxv, Act.Tanh)
            pxt = tps.tile([128, 1024], BF16, tag="pxt")
            mm(pxt[:, 0:32], xp_s[:, 0:128], identb[:],
               is_transpose=True, start=True, stop=False, skip_group_check=True)
            mm(pxt[:, 32:64], xp_s[:, 128:256], identb[:],
               is_transpose=True, start=False, stop=True, skip_group_check=True)
            xT_sb = work.tile([128, 64], BF16, tag="xT_sb")
            nc.vector.tensor_copy(xT_sb[:], pxt[:, 0:64])

            # ---- 8 gate x-matmuls: cols 0..383 = gxn|r|z ----
            for cx in range(2):
                for j in range(4):
                    mm(g[32 * j : 32 * (j + 1), 0:384],
                       xT_sb[:, 32 * cx : 32 * (cx + 1)],
                       wih_sb[:, GATE3 * cx + 384 * j : GATE3 * cx + 384 * (j + 1)],
                       start=False, stop=(cx == 1), skip_group_check=True,
                       tile_position=(0, 32 * j))

            # ---- gate nonlinearity + state update ----
            rz = work.tile([128, 256], BF16, tag="rz")
            nc.scalar.activation(rz[:, 0:128], g[:, 128:256], Act.Sigmoid)
            t1 = work.tile([128, 128], BF16, tag="t1")
            nc.vector.tensor_mul(t1[:], rz[:, 0:128], g[:, 384:512])
            t2 = work.tile([128, 128], F32, tag="t2")
            nc.vector.tensor_add(t2[:], t1[:], g[:, 0:128])
            nc.scalar.activation(rz[:, 128:256], g[:, 256:384], Act.Sigmoid)
            n_s = work.tile([128, 128], BF16, tag="n_s")
            nc.scalar.activation(n_s[:], t2[:], Act.Tanh)
            d_s = work.tile([128, 128], BF16, tag="d_s")
            nc.vector.tensor_sub(d_s[:], h_sb[:], n_s[:])
            p_s = work.tile([128, 128], BF16, tag="p_s")
            nc.vector.tensor_mul(p_s[:], rz[:, 128:256], d_s[:])
            nc.vector.tensor_add(h_sb[:], n_s[:], p_s[:])

            # ---- hT via DVE 32x32 block transpose (bf16 -> bf16) ----
            nc.vector.transpose(hT_sb[:], h_sb[:])

        nc.sync.dma_start(out=hout.ap(), in_=h_sb[:])


_CACHE = {}


def _get_nc():
    if "nc" not in _CACHE:
        _CACHE["nc"] = _build_nc()
    return _CACHE["nc"]


def _prep_shared(W_ih, W_hh, b_ih, b_hh, Wt_h, Wt_x, bt):
    k = np.arange(128)
    # DVE block-transpose layout: chunk c, partition k -> h dim 128*(k//32)+32*c+(k%32)
    perm = [128 * (k // 32) + 32 * c + (k % 32) for c in range(4)]

    wtxa = np.ascontiguousarray(Wt_x.T[0:128, :]).astype(BF)  # [128, 256]
    wtxb = np.ascontiguousarray(
        np.vstack([Wt_x.T[128:160, :], bt[None, :]])
    ).astype(BF)  # [33, 256]

    wth = np.zeros((128, 4 * XP), np.float32)
    for c in range(4):
        wth[:, XP * c : XP * (c + 1)] = Wt_h[:, perm[c]].T
    wth = wth.astype(BF)

    # gate row orders per hidden-block j
    rows_h, rows_x = [], []
    for j in range(4):
        rj = np.arange(128 * j, 128 * (j + 1))
        rows_h.append(np.concatenate([rj, H + rj, 2 * H + rj]))        # r|z|ghn
        rows_x.append(np.concatenate([2 * H + rj, rj, H + rj]))        # gxn|r|z
    rows_h = np.concatenate(rows_h)  # [1536]
    rows_x = np.concatenate(rows_x)  # [1536]

    whh = np.zeros((128, 4 * GATE3), np.float32)
    for c in range(4):
        whh[:, GATE3 * c : GATE3 * (c + 1)] = W_hh[rows_h][:, perm[c]].T
    whh = whh.astype(BF)

    wih = np.zeros((128, 2 * GATE3), np.float32)
    for cx in range(2):
        wih[:, GATE3 * cx : GATE3 * (cx + 1)] = (
            W_ih[rows_x][:, 128 * cx : 128 * (cx + 1)].T
        )
    wih = wih.astype(BF)

    blhs = np.zeros((4, 128), np.float32)
    for j in range(4):
        blhs[j, 32 * j : 32 * (j + 1)] = 1.0
    brhs = np.zeros((4, 512), np.float32)
    for j in range(4):
        rj = np.arange(128 * j, 128 * (j + 1))
        brhs[j, 0:128] = b_ih[2 * H + rj]
        brhs[j, 128:256] = b_ih[rj] + b_hh[rj]
        brhs[j, 256:384] = b_ih[H + rj] + b_hh[H + rj]
        brhs[j, 384:512] = b_hh[2 * H + rj]

    return wtxa, wtxb, wth, whh, wih, blhs, brhs


def _make_in_maps(x_l_seq, x_t_seq, x_w_seq, x_s_seq, shared):
    wtxa, wtxb, wth, whh, wih, blhs, brhs = shared
    x_cat = np.concatenate(
        [np.asarray(x_l_seq), np.asarray(x_t_seq), np.asarray(x_w_seq),
         np.asarray(x_s_seq)],
        axis=-1,
    ).astype(np.float32)  # [B, T, 160]
    in_maps = []
    for c in range(NCORES):
        xc = x_cat[BC * c : BC * (c + 1)]  # [32, T, 160]
        xTc = xc.transpose(2, 1, 0).reshape(D, T * BC)  # [160, t*32+b]
        xTa = np.vstack([xTc, np.ones((1, T * BC), np.float32)]).astype(BF)
        in_maps.append(
            {
                "xT": np.ascontiguousarray(xTa),
                "wtxa": wtxa,
                "wtxb": wtxb,
                "wth": wth,
                "whh": whh,
                "wih": wih,
                "blhs": blhs,
                "brhs": brhs,
            }
        )
    return in_maps


def kernel(x_l_seq, x_t_seq, x_w_seq, x_s_seq, W_ih, W_hh, b_ih, b_hh, Wt_h, Wt_x, bt):
    nc = _get_nc()
    shared = _prep_shared(
        np.asarray(W_ih, np.float32), np.asarray(W_hh, np.float32),
        np.asarray(b_ih, np.float32), np.asarray(b_hh, np.float32),
        np.asarray(Wt_h, np.float32), np.asarray(Wt_x, np.float32),
        np.asarray(bt, np.float32),
    )
    in_maps = _make_in_maps(x_l_seq, x_t_seq, x_w_seq, x_s_seq, shared)
    res = run_bass_kernel_spmd(nc, in_maps, core_ids=list(range(NCORES)))
    out = np.zeros((1, B, H), np.float32)
    for c in range(NCORES):
        hc = np.asarray(res.results[c]["hout"], np.float32)  # [128, 128]
        out[0, BC * c : BC * (c + 1), :] = (
            hc.reshape(4, 32, 128).transpose(1, 0, 2).reshape(32, H)
        )
    return out
